# revision 35
# baseline (speedup 1.0000x reference)
"""Trainium2 Bass kernel for AttentiveGraphConvolutionSparse (GAT-style layer).

Computation (see reference):
    h   = x @ W                         [N, D_OUT]
    a_s = h @ attn_self                 [N, 1]
    a_n = h @ attn_neigh                [N, 1]
    e   = leaky_relu(a_s + a_n.T, 0.2)  [N, N]
    e  += MASK_VAL * (1 - adj)
    attn = softmax(e, axis=-1)
    out = relu(attn @ h)

Sharding: row-parallel over queries. Core c owns rows [c*R, (c+1)*R) of adj
and of the output; W / attn vectors replicated; h/a_n all-gathered (computed
from the local x slice, then exchanged via AllGather collectives).

Key implementation ideas:
  - softmax needs no max-subtraction (logits are bounded, fp32 exp is safe);
    masked entries are handled by adding adj to leaky(t)/BIG during the adj
    DMA itself (CCE accumulate), then computing exp(BIG*z - BIG):
        unmasked (adj=1): exp(leaky(t))      masked (adj=0): exp(leaky(t)-BIG) ~= 0
  - the big attn@h matmul contracts over j (neighbors), which must sit on
    partitions; adj arrives row-major (i on partitions), so pre-exp logits are
    PE-transposed (f32) and the exp reads PSUM and writes transposed bf16
    tiles that feed the matmul directly (the PSUM->SBUF copy IS the exp).
  - denominators come free as a ones-column appended to h.
"""

import os

import numpy as np

import concourse.bacc as bacc
import concourse.bass as bass
import concourse.mybir as mybir
import concourse.tile as tile
from concourse import masks
from concourse.bass_utils import run_bass_kernel_spmd
from concourse.mybir import ActivationFunctionType as AFT, AluOpType as ALU

N = 12288
D_IN = 512
D_OUT = 256
NCORES = 8
R = N // NCORES          # 1536 query rows per core
P = 128
NIT = R // P             # 12 i-tiles per core
NJT = N // P             # 96 j-tiles
GROUPS = 3
ITG = NIT // GROUPS      # 4 i-tiles per group (PSUM-resident accumulators)
QW = 1024                # j-quarter width processed per z tile
NJQ = N // QW            # 12 quarters
JLQ = QW // P            # 8 j-tiles per quarter
BIG = 100.0
HCOL = D_OUT + 1         # h plus ones column (softmax denominator)
F32 = mybir.dt.float32
BF16 = mybir.dt.bfloat16

_cache = {}
last_results = None


def _register_leaky_mask():
    """Custom DVE op: out = max(y, s1*y)*imm2 + in1, with y = in0 + s0.

    One Vector pass computes the whole masked-logit tile:
        z = leaky_relu(a_n_bcast + a_s, alpha)/BIG + adj
    (in0 = a_n broadcast, s0 = a_s per-partition, s1 = alpha, imm2 = 1/BIG,
     in1 = adj tile). exp(BIG*z - BIG) then yields adj ? exp(leaky) : ~0.
    """
    from concourse import dve_ops as dops
    from concourse.dve_spec import (
        C0, C1, C2, Spec, Src0, Src1, lower as dve_lower, maxx,
    )
    from concourse.dve_uop import DveOpSpec

    name = "LEAKY_MASK_ANT"
    for op in dops.OPS:
        if op.name == name:
            return op
    _y = Src0 + C0
    spec = Spec(
        body=maxx(_y, _y * C1) * C2 + Src1,
        reference=lambda in0, in1, s0, s1, imm2: (
            np.maximum(in0.astype(np.float32) + s0,
                       (in0.astype(np.float32) + s0) * s1) * imm2 + in1
        ).astype(np.float32),
    )
    row = dops._CUSTOM_DVE_ROW_BASE + len(dops.OPS)
    assert row < 0x20
    sha = {}
    for ver in ("v3", "v4"):
        s = DveOpSpec(name=name, opcode=row, uops=dve_lower(spec, ver=ver),
                      rd1_en=True)
        sha[ver] = s.sha(ver)
    op = dops.DveOp(name, spec, subdim=False, uops_sha=sha)
    dops.OPS.append(op)
    dops._SUB_OPCODE_FOR_NAME[name] = row
    dops.CUSTOM_DVE_SPECS[name] = spec
    return op


def _install_ntff_hook():
    """Register the axon NTFF profiling hook (missing antenv.axon_hooks shim).

    Only used when KERNEL_TRACE is set (dev profiling); replicates
    trn_boot._ntff_profile_via_ctypes against /opt/axon/libaxon_pjrt.so.
    """
    import contextlib
    import ctypes
    import sys
    import types

    if "antenv.axon_hooks" in sys.modules:
        return
    so_path = "/opt/axon/libaxon_pjrt.so"
    if not os.path.exists(so_path):
        return
    lib = ctypes.CDLL(so_path)
    if not hasattr(lib, "axon_start_nrt_profile"):
        return
    lib.axon_start_nrt_profile.argtypes = [ctypes.POINTER(ctypes.c_int64),
                                           ctypes.c_size_t]
    lib.axon_start_nrt_profile.restype = ctypes.c_int64
    lib.axon_stop_nrt_profile.argtypes = [ctypes.c_char_p]
    lib.axon_stop_nrt_profile.restype = ctypes.c_int64

    @contextlib.contextmanager
    def _hook(output_dir, device_ids):
        import jax
        jax.devices()
        if device_ids:
            ids = (ctypes.c_int64 * len(device_ids))(*device_ids)
            rc = lib.axon_start_nrt_profile(ids, len(device_ids))
        else:
            rc = lib.axon_start_nrt_profile(None, 0)
        if rc != 0:
            raise RuntimeError(f"axon_start_nrt_profile rc={rc}")
        try:
            yield
        finally:
            n = lib.axon_stop_nrt_profile(str(output_dir).encode())
            print(f"profile: {n} file(s) written to {output_dir}",
                  file=sys.stderr)

    _state = {"hook": _hook}
    mod = types.ModuleType("antenv.axon_hooks")
    mod.get_axon_ntff_profile_hook = lambda: _state["hook"]

    def _set(h):
        _state["hook"] = h

    mod.set_axon_ntff_profile_hook = _set
    sys.modules["antenv.axon_hooks"] = mod


def _build():
    global _LEAKY_MASK
    _LEAKY_MASK = _register_leaky_mask()
    nc = bacc.Bacc("TRN2", target_bir_lowering=False, debug=False,
                   num_devices=NCORES)

    x_p = nc.declare_dram_parameter("x", [R, D_IN], F32, isOutput=False).ap()
    w_p = nc.declare_dram_parameter("W", [D_IN, D_OUT], F32, isOutput=False).ap()
    as_p = nc.declare_dram_parameter("attn_self", [D_OUT, 1], F32, isOutput=False).ap()
    an_p = nc.declare_dram_parameter("attn_neigh", [D_OUT, 1], F32, isOutput=False).ap()
    adj_p = nc.declare_dram_parameter("adj", [R, N], F32, isOutput=False).ap()
    out_p = nc.declare_dram_parameter("out", [R, D_OUT], F32, isOutput=True).ap()

    h_loc = nc.dram_tensor("h_loc", [R, D_OUT], BF16).ap()
    h_all = nc.dram_tensor("h_all", [N, D_OUT], BF16, addr_space="Shared").ap()
    an_loc = nc.dram_tensor("an_loc", [R, 1], F32).ap()
    an_all = nc.dram_tensor("an_all", [N, 1], F32, addr_space="Shared").ap()
    warm_loc = nc.dram_tensor("warm_loc", [1, 1], F32).ap()
    warm_all = nc.dram_tensor("warm_all", [NCORES, 1], F32,
                              addr_space="Shared").ap()

    from contextlib import ExitStack

    with tile.TileContext(nc) as tc, ExitStack() as octx:
        const = octx.enter_context(tc.tile_pool(name="const", bufs=1))
        ident = const.tile([P, P], F32)
        masks.make_identity(nc, ident[:])
        ident_bf = const.tile([P, P], BF16)
        masks.make_identity(nc, ident_bf[:])

        h_big = const.tile([P, NJT * HCOL], BF16)
        anb = const.tile([P, N], F32)
        as_sb = const.tile([P, NIT], F32)
        negbig = const.tile([P, 1], F32)
        nc.gpsimd.memset(negbig[:], -BIG)

        # ---------------- prologue: h, a_s, a_n ----------------
        with tc.tile_pool(name="prol", bufs=1) as prol, \
             tc.tile_pool(name="prps", bufs=2, space="PSUM") as prps, \
             tc.tile_pool(name="prsb", bufs=3) as prsb:
            # loads on the sync ring, emitted before the adj stream (FIFO)
            xnat = prol.tile([P, NIT * D_IN], F32)
            nc.sync.dma_start(
                out=xnat[:].rearrange("p (it k) -> p it k", it=NIT),
                in_=x_p.rearrange("(it p) k -> p it k", p=P))
            wnat = prol.tile([P, 4 * D_OUT], F32)
            nc.sync.dma_start(
                out=wnat[:].rearrange("p (kb d) -> p kb d", kb=4),
                in_=w_p.rearrange("(kb p) d -> p kb d", p=P))
            asn = prol.tile([P, 4], F32)
            nc.sync.dma_start(
                out=asn[:, 0:2].rearrange("p (db one) -> p db one", one=1),
                in_=as_p.rearrange("(db p) one -> p db one", p=P))
            nc.sync.dma_start(
                out=asn[:, 2:4].rearrange("p (db one) -> p db one", one=1),
                in_=an_p.rearrange("(db p) one -> p db one", p=P))

            # x^T tiles: col kb*R + it*P + i
            xT = prol.tile([P, 4 * R], F32)
            for kb in range(4):
                for it in range(NIT):
                    pst = prps.tile([P, P], F32, tag="pst")
                    nc.tensor.transpose(
                        pst[:], xnat[:, it * D_IN + kb * P: it * D_IN + (kb + 1) * P],
                        ident[:])
                    nc.vector.tensor_copy(
                        xT[:, kb * R + it * P: kb * R + (it + 1) * P], pst[:])

            # W^T tiles: col db*D_IN + kb*P + k
            wT = prol.tile([P, 2 * D_IN], F32)
            for kb in range(4):
                for db in range(2):
                    pst = prps.tile([P, P], F32, tag="pst")
                    nc.tensor.transpose(
                        pst[:], wnat[:, kb * D_OUT + db * P: kb * D_OUT + (db + 1) * P],
                        ident[:])
                    nc.vector.tensor_copy(
                        wT[:, db * D_IN + kb * P: db * D_IN + (kb + 1) * P], pst[:])

            # w_s / w_n = W @ attn_{self,neigh}, packed next to W columns:
            # wext cols per kb: [ W_kb (256) | w_s_kb | w_n_kb ]  (258 wide)
            wext = prol.tile([P, 4 * (D_OUT + 2)], F32)
            for kb in range(4):
                nc.vector.tensor_copy(
                    wext[:, kb * (D_OUT + 2): kb * (D_OUT + 2) + D_OUT],
                    wnat[:, kb * D_OUT: (kb + 1) * D_OUT])
            for v in range(2):
                for kb in range(4):
                    wps = prps.tile([P, 1], F32, tag="wps")
                    for db in range(2):
                        nc.tensor.matmul(
                            wps[:],
                            wT[:, db * D_IN + kb * P: db * D_IN + (kb + 1) * P],
                            asn[:, 2 * v + db: 2 * v + db + 1],
                            start=(db == 0), stop=(db == 1))
                    nc.vector.tensor_copy(
                        wext[:, kb * (D_OUT + 2) + D_OUT + v:
                              kb * (D_OUT + 2) + D_OUT + v + 1], wps[:])

            grp = [list(range(NCORES))]

            # a_s / a_n first (N=2 matmuls) - they gate the whole leaky/exp
            # pipeline via the a_n AllGather, h only gates the matmuls
            an_sb = prol.tile([P, NIT], F32)
            for it in range(NIT):
                aps = prps.tile([P, 2], F32, tag="aps")
                for kb in range(4):
                    nc.tensor.matmul(
                        aps[:],
                        xT[:, kb * R + it * P: kb * R + (it + 1) * P],
                        wext[:, kb * (D_OUT + 2) + D_OUT:
                             (kb + 1) * (D_OUT + 2)],
                        start=(kb == 0), stop=(kb == 3))
                nc.vector.tensor_copy(as_sb[:, it:it + 1], aps[:, 0:1])
                nc.vector.tensor_copy(an_sb[:, it:it + 1], aps[:, 1:2])
            nc.scalar.dma_start(
                out=an_loc.rearrange("(it p) one -> p it one", p=P),
                in_=an_sb[:].rearrange("p (it one) -> p it one", one=1))
            nc.gpsimd.collective_compute(
                "AllGather", ALU.bypass, replica_groups=grp,
                ins=[an_loc[:]], outs=[an_all[:]])

            # h slices -> local buffer, then AllGather
            for it in range(NIT):
                hps = prps.tile([P, D_OUT], F32, tag="hps")
                for kb in range(4):
                    nc.tensor.matmul(
                        hps[:],
                        xT[:, kb * R + it * P: kb * R + (it + 1) * P],
                        wext[:, kb * (D_OUT + 2): kb * (D_OUT + 2) + D_OUT],
                        start=(kb == 0), stop=(kb == 3))
                hsb = prsb.tile([P, D_OUT], BF16, tag="hsb")
                nc.vector.tensor_copy(hsb[:], hps[:])
                nc.scalar.dma_start(out=h_loc[it * P:(it + 1) * P, :],
                                    in_=hsb[:])
            nc.gpsimd.collective_compute(
                "AllGather", ALU.bypass, replica_groups=grp,
                ins=[h_loc[:]], outs=[h_all[:]])

        # gathered h -> SBUF with ones column (gpsimd ring: keeps the
        # collective-gated DMA off the scalar ring so queued exps don't stall;
        # chunked by jq so the first matmuls start on the first chunk)
        h_big_v = h_big[:].rearrange("p (jt c) -> p jt c", jt=NJT)
        h_all_v = h_all.rearrange("(jt p) d -> p jt d", p=P)
        for jq in range(NJQ):
            nc.gpsimd.dma_start(
                out=h_big_v[:, jq * JLQ:(jq + 1) * JLQ, 0:D_OUT],
                in_=h_all_v[:, jq * JLQ:(jq + 1) * JLQ, :])
        nc.vector.memset(h_big_v[:, :, D_OUT:HCOL], 1.0)

        # a_n broadcast across partitions (0-stride partition read from DRAM),
        # chunked so the first quarter's leaky isn't gated on the full 6.3MB
        an_bcast_src = an_all.rearrange("n one -> one n").to_broadcast((P, N))
        for jq in range(NJQ):
            nc.gpsimd.dma_start(
                out=anb[:, jq * QW:(jq + 1) * QW],
                in_=an_bcast_src[:, jq * QW:(jq + 1) * QW])

        # ---------------- main attention loop ----------------
        zpools = [octx.enter_context(tc.tile_pool(name=f"z{i}", bufs=3))
                  for i in range(ITG)]
        spool = octx.enter_context(tc.tile_pool(name="spool", bufs=3))
        mpsum = octx.enter_context(tc.tile_pool(name="mpsum", bufs=4, space="PSUM"))
        opsum = octx.enter_context(tc.tile_pool(name="opsum", bufs=1, space="PSUM"))
        fpool = octx.enter_context(tc.tile_pool(name="fpool", bufs=2))

        for g in range(GROUPS):
            outps = [opsum.tile([P, HCOL], F32, name=f"outp{i6}", tag=f"outp{i6}")
                     for i6 in range(ITG)]
            for jq in range(NJQ):
                sns = []
                for i6 in range(ITG):
                    it = g * ITG + i6
                    zt = zpools[i6].tile([P, QW], F32)
                    # adj rows, plain line-rate DMA (sync HWDGE ring)
                    nc.sync.dma_start(
                        out=zt[:],
                        in_=adj_p[it * P:(it + 1) * P, jq * QW:(jq + 1) * QW])
                    # in-place fused: zt = leaky(anb + a_s)/BIG + zt(adj)
                    nc.vector._custom_dve(
                        _LEAKY_MASK, out=zt[:],
                        in0=anb[:, jq * QW:(jq + 1) * QW], in1=zt[:],
                        s0=as_sb[:, it:it + 1], s1=0.2, imm2=1.0 / BIG)
                    # s = exp(BIG*z - BIG) natural layout, bf16
                    sn = spool.tile([P, QW], BF16, tag=f"sn{i6}")
                    nc.scalar.activation(out=sn[:], in_=zt[:], func=AFT.Exp,
                                         bias=negbig[:], scale=BIG)
                    sns.append(sn)
                for jl in range(JLQ):
                    jt = jq * JLQ + jl
                    zps = mpsum.tile([P, ITG * P], BF16)
                    for i6 in range(ITG):
                        nc.tensor.transpose(
                            zps[:, i6 * P:(i6 + 1) * P],
                            sns[i6][:, jl * P:(jl + 1) * P], ident_bf[:])
                    st = spool.tile([P, ITG * P], BF16, tag="st", bufs=4)
                    # PSUM -> SBUF copy, split over the engines with slack
                    if jl % 8 < 5:
                        nc.scalar.copy(st[:], zps[:])
                    else:
                        nc.vector.tensor_copy(st[:], zps[:])
                    for i6 in range(ITG):
                        nc.tensor.matmul(
                            outps[i6][:],
                            st[:, i6 * P:(i6 + 1) * P],
                            h_big[:, jt * HCOL:(jt + 1) * HCOL],
                            start=(jq == 0 and jl == 0),
                            stop=(jq == NJQ - 1 and jl == JLQ - 1))
            for i6 in range(ITG):
                it = g * ITG + i6
                rec = fpool.tile([P, 1], F32, tag="rec")
                nc.vector.reciprocal(rec[:], outps[i6][:, D_OUT:HCOL])
                of = fpool.tile([P, D_OUT], F32, tag="of")
                nc.vector.tensor_scalar(
                    out=of[:], in0=outps[i6][:, 0:D_OUT],
                    scalar1=rec[:], scalar2=0.0,
                    op0=ALU.mult, op1=ALU.max)
                nc.gpsimd.dma_start(out=out_p[it * P:(it + 1) * P, :], in_=of[:])

    nc.compile()
    return nc


def kernel(x, W, attn_self, attn_neigh, adj):
    global last_results
    if "nc" not in _cache:
        _cache["nc"] = _build()
    nc = _cache["nc"]

    x = np.ascontiguousarray(np.asarray(x, dtype=np.float32))
    W = np.ascontiguousarray(np.asarray(W, dtype=np.float32))
    attn_self = np.ascontiguousarray(np.asarray(attn_self, dtype=np.float32))
    attn_neigh = np.ascontiguousarray(np.asarray(attn_neigh, dtype=np.float32))
    adj = np.asarray(adj, dtype=np.float32)

    in_maps = []
    for c in range(NCORES):
        sl = slice(c * R, (c + 1) * R)
        in_maps.append({
            "x": np.ascontiguousarray(x[sl]),
            "W": W,
            "attn_self": attn_self,
            "attn_neigh": attn_neigh,
            "adj": np.ascontiguousarray(adj[sl]),
        })

    trace = bool(os.environ.get("KERNEL_TRACE"))
    if trace:
        _install_ntff_hook()
    res = run_bass_kernel_spmd(nc, in_maps, list(range(NCORES)), trace=trace)
    last_results = res
    return np.concatenate([res.results[c]["out"] for c in range(NCORES)], axis=0)


# revision 36
# speedup vs baseline: 1.0653x; 1.0653x over previous
"""Trainium2 Bass kernel for AttentiveGraphConvolutionSparse (GAT-style layer).

Computation (see reference):
    h   = x @ W                         [N, D_OUT]
    a_s = h @ attn_self                 [N, 1]
    a_n = h @ attn_neigh                [N, 1]
    e   = leaky_relu(a_s + a_n.T, 0.2)  [N, N]
    e  += MASK_VAL * (1 - adj)
    attn = softmax(e, axis=-1)
    out = relu(attn @ h)

Sharding: row-parallel over queries. Core c owns rows [c*R, (c+1)*R) of adj
and of the output; W / attn vectors replicated; h/a_n all-gathered (computed
from the local x slice, then exchanged via AllGather collectives).

Key implementation ideas:
  - softmax needs no max-subtraction (logits are bounded, fp32 exp is safe);
    masked entries are handled by adding adj to leaky(t)/BIG during the adj
    DMA itself (CCE accumulate), then computing exp(BIG*z - BIG):
        unmasked (adj=1): exp(leaky(t))      masked (adj=0): exp(leaky(t)-BIG) ~= 0
  - the big attn@h matmul contracts over j (neighbors), which must sit on
    partitions; adj arrives row-major (i on partitions), so pre-exp logits are
    PE-transposed (f32) and the exp reads PSUM and writes transposed bf16
    tiles that feed the matmul directly (the PSUM->SBUF copy IS the exp).
  - denominators come free as a ones-column appended to h.
"""

import os

import numpy as np

import concourse.bacc as bacc
import concourse.bass as bass
import concourse.mybir as mybir
import concourse.tile as tile
from concourse import masks
from concourse.bass_utils import run_bass_kernel_spmd
from concourse.mybir import ActivationFunctionType as AFT, AluOpType as ALU

N = 12288
D_IN = 512
D_OUT = 256
NCORES = 8
R = N // NCORES          # 1536 query rows per core
P = 128
NIT = R // P             # 12 i-tiles per core
NJT = N // P             # 96 j-tiles
GROUPS = 3
ITG = NIT // GROUPS      # 4 i-tiles per group (PSUM-resident accumulators)
QW = 1024                # j-quarter width processed per z tile
NJQ = N // QW            # 12 quarters
JLQ = QW // P            # 8 j-tiles per quarter
BIG = 100.0
HCOL = D_OUT + 1         # h plus ones column (softmax denominator)
F32 = mybir.dt.float32
BF16 = mybir.dt.bfloat16

_cache = {}
last_results = None


def _register_leaky_mask():
    """Custom DVE op: out = max(y, s1*y)*imm2 + in1, with y = in0 + s0.

    One Vector pass computes the whole masked-logit tile:
        z = leaky_relu(a_n_bcast + a_s, alpha)/BIG + adj
    (in0 = a_n broadcast, s0 = a_s per-partition, s1 = alpha, imm2 = 1/BIG,
     in1 = adj tile). exp(BIG*z - BIG) then yields adj ? exp(leaky) : ~0.
    """
    from concourse import dve_ops as dops
    from concourse.dve_spec import (
        C0, C1, C2, Spec, Src0, Src1, lower as dve_lower, maxx,
    )
    from concourse.dve_uop import DveOpSpec

    name = "LEAKY_MASK_ANT"
    for op in dops.OPS:
        if op.name == name:
            return op
    _y = Src0 + C0
    spec = Spec(
        body=maxx(_y, _y * C1) * C2 + Src1,
        reference=lambda in0, in1, s0, s1, imm2: (
            np.maximum(in0.astype(np.float32) + s0,
                       (in0.astype(np.float32) + s0) * s1) * imm2 + in1
        ).astype(np.float32),
    )
    row = dops._CUSTOM_DVE_ROW_BASE + len(dops.OPS)
    assert row < 0x20
    sha = {}
    for ver in ("v3", "v4"):
        s = DveOpSpec(name=name, opcode=row, uops=dve_lower(spec, ver=ver),
                      rd1_en=True)
        sha[ver] = s.sha(ver)
    op = dops.DveOp(name, spec, subdim=False, uops_sha=sha)
    dops.OPS.append(op)
    dops._SUB_OPCODE_FOR_NAME[name] = row
    dops.CUSTOM_DVE_SPECS[name] = spec
    return op


def _install_ntff_hook():
    """Register the axon NTFF profiling hook (missing antenv.axon_hooks shim).

    Only used when KERNEL_TRACE is set (dev profiling); replicates
    trn_boot._ntff_profile_via_ctypes against /opt/axon/libaxon_pjrt.so.
    """
    import contextlib
    import ctypes
    import sys
    import types

    if "antenv.axon_hooks" in sys.modules:
        return
    so_path = "/opt/axon/libaxon_pjrt.so"
    if not os.path.exists(so_path):
        return
    lib = ctypes.CDLL(so_path)
    if not hasattr(lib, "axon_start_nrt_profile"):
        return
    lib.axon_start_nrt_profile.argtypes = [ctypes.POINTER(ctypes.c_int64),
                                           ctypes.c_size_t]
    lib.axon_start_nrt_profile.restype = ctypes.c_int64
    lib.axon_stop_nrt_profile.argtypes = [ctypes.c_char_p]
    lib.axon_stop_nrt_profile.restype = ctypes.c_int64

    @contextlib.contextmanager
    def _hook(output_dir, device_ids):
        import jax
        jax.devices()
        if device_ids:
            ids = (ctypes.c_int64 * len(device_ids))(*device_ids)
            rc = lib.axon_start_nrt_profile(ids, len(device_ids))
        else:
            rc = lib.axon_start_nrt_profile(None, 0)
        if rc != 0:
            raise RuntimeError(f"axon_start_nrt_profile rc={rc}")
        try:
            yield
        finally:
            n = lib.axon_stop_nrt_profile(str(output_dir).encode())
            print(f"profile: {n} file(s) written to {output_dir}",
                  file=sys.stderr)

    _state = {"hook": _hook}
    mod = types.ModuleType("antenv.axon_hooks")
    mod.get_axon_ntff_profile_hook = lambda: _state["hook"]

    def _set(h):
        _state["hook"] = h

    mod.set_axon_ntff_profile_hook = _set
    sys.modules["antenv.axon_hooks"] = mod


def _build():
    global _LEAKY_MASK
    _LEAKY_MASK = _register_leaky_mask()
    nc = bacc.Bacc("TRN2", target_bir_lowering=False, debug=False,
                   num_devices=NCORES)

    x_p = nc.declare_dram_parameter("x", [R, D_IN], F32, isOutput=False).ap()
    w_p = nc.declare_dram_parameter("W", [D_IN, D_OUT], F32, isOutput=False).ap()
    as_p = nc.declare_dram_parameter("attn_self", [D_OUT, 1], F32, isOutput=False).ap()
    an_p = nc.declare_dram_parameter("attn_neigh", [D_OUT, 1], F32, isOutput=False).ap()
    adj_p = nc.declare_dram_parameter("adj", [R, N], F32, isOutput=False).ap()
    out_p = nc.declare_dram_parameter("out", [R, D_OUT], F32, isOutput=True).ap()

    h_loc = nc.dram_tensor("h_loc", [R, D_OUT], BF16).ap()
    h_all = nc.dram_tensor("h_all", [N, D_OUT], BF16, addr_space="Shared").ap()
    an_loc = nc.dram_tensor("an_loc", [R, 1], F32).ap()
    an_all = nc.dram_tensor("an_all", [N, 1], F32, addr_space="Shared").ap()
    warm_loc = nc.dram_tensor("warm_loc", [1, 1], F32).ap()
    warm_all = nc.dram_tensor("warm_all", [NCORES, 1], F32,
                              addr_space="Shared").ap()

    from contextlib import ExitStack

    with tile.TileContext(nc) as tc, ExitStack() as octx:
        const = octx.enter_context(tc.tile_pool(name="const", bufs=1))
        # fire a dummy collective immediately: the ncfw collective engine has
        # long first-use startup latency; this hides it under the prologue
        nc.gpsimd.collective_compute(
            "AllGather", ALU.bypass, replica_groups=[list(range(NCORES))],
            ins=[warm_loc[:]], outs=[warm_all[:]])
        ident = const.tile([P, P], F32)
        masks.make_identity(nc, ident[:])
        ident_bf = const.tile([P, P], BF16)
        masks.make_identity(nc, ident_bf[:])

        h_big = const.tile([P, NJT * HCOL], BF16)
        anb = const.tile([P, N], F32)
        as_sb = const.tile([P, NIT], F32)
        negbig = const.tile([P, 1], F32)
        nc.gpsimd.memset(negbig[:], -BIG)

        # ---------------- prologue: h, a_s, a_n ----------------
        with tc.tile_pool(name="prol", bufs=1) as prol, \
             tc.tile_pool(name="prps", bufs=2, space="PSUM") as prps, \
             tc.tile_pool(name="prsb", bufs=3) as prsb:
            # loads on the sync ring, emitted before the adj stream (FIFO)
            xnat = prol.tile([P, NIT * D_IN], F32)
            nc.sync.dma_start(
                out=xnat[:].rearrange("p (it k) -> p it k", it=NIT),
                in_=x_p.rearrange("(it p) k -> p it k", p=P))
            wnat = prol.tile([P, 4 * D_OUT], F32)
            nc.sync.dma_start(
                out=wnat[:].rearrange("p (kb d) -> p kb d", kb=4),
                in_=w_p.rearrange("(kb p) d -> p kb d", p=P))
            asn = prol.tile([P, 4], F32)
            nc.sync.dma_start(
                out=asn[:, 0:2].rearrange("p (db one) -> p db one", one=1),
                in_=as_p.rearrange("(db p) one -> p db one", p=P))
            nc.sync.dma_start(
                out=asn[:, 2:4].rearrange("p (db one) -> p db one", one=1),
                in_=an_p.rearrange("(db p) one -> p db one", p=P))

            # x^T tiles: col kb*R + it*P + i
            xT = prol.tile([P, 4 * R], F32)
            for kb in range(4):
                for it in range(NIT):
                    pst = prps.tile([P, P], F32, tag="pst")
                    nc.tensor.transpose(
                        pst[:], xnat[:, it * D_IN + kb * P: it * D_IN + (kb + 1) * P],
                        ident[:])
                    nc.vector.tensor_copy(
                        xT[:, kb * R + it * P: kb * R + (it + 1) * P], pst[:])

            # W^T tiles: col db*D_IN + kb*P + k
            wT = prol.tile([P, 2 * D_IN], F32)
            for kb in range(4):
                for db in range(2):
                    pst = prps.tile([P, P], F32, tag="pst")
                    nc.tensor.transpose(
                        pst[:], wnat[:, kb * D_OUT + db * P: kb * D_OUT + (db + 1) * P],
                        ident[:])
                    nc.vector.tensor_copy(
                        wT[:, db * D_IN + kb * P: db * D_IN + (kb + 1) * P], pst[:])

            # w_s / w_n = W @ attn_{self,neigh}, packed next to W columns:
            # wext cols per kb: [ W_kb (256) | w_s_kb | w_n_kb ]  (258 wide)
            wext = prol.tile([P, 4 * (D_OUT + 2)], F32)
            for kb in range(4):
                nc.vector.tensor_copy(
                    wext[:, kb * (D_OUT + 2): kb * (D_OUT + 2) + D_OUT],
                    wnat[:, kb * D_OUT: (kb + 1) * D_OUT])
            for v in range(2):
                for kb in range(4):
                    wps = prps.tile([P, 1], F32, tag="wps")
                    for db in range(2):
                        nc.tensor.matmul(
                            wps[:],
                            wT[:, db * D_IN + kb * P: db * D_IN + (kb + 1) * P],
                            asn[:, 2 * v + db: 2 * v + db + 1],
                            start=(db == 0), stop=(db == 1))
                    nc.vector.tensor_copy(
                        wext[:, kb * (D_OUT + 2) + D_OUT + v:
                              kb * (D_OUT + 2) + D_OUT + v + 1], wps[:])

            grp = [list(range(NCORES))]

            # a_s / a_n first (N=2 matmuls) - they gate the whole leaky/exp
            # pipeline via the a_n AllGather, h only gates the matmuls
            an_sb = prol.tile([P, NIT], F32)
            for it in range(NIT):
                aps = prps.tile([P, 2], F32, tag="aps")
                for kb in range(4):
                    nc.tensor.matmul(
                        aps[:],
                        xT[:, kb * R + it * P: kb * R + (it + 1) * P],
                        wext[:, kb * (D_OUT + 2) + D_OUT:
                             (kb + 1) * (D_OUT + 2)],
                        start=(kb == 0), stop=(kb == 3))
                nc.vector.tensor_copy(as_sb[:, it:it + 1], aps[:, 0:1])
                nc.vector.tensor_copy(an_sb[:, it:it + 1], aps[:, 1:2])
            nc.scalar.dma_start(
                out=an_loc.rearrange("(it p) one -> p it one", p=P),
                in_=an_sb[:].rearrange("p (it one) -> p it one", one=1))
            nc.gpsimd.collective_compute(
                "AllGather", ALU.bypass, replica_groups=grp,
                ins=[an_loc[:]], outs=[an_all[:]])

            # h slices -> local buffer, then AllGather
            for it in range(NIT):
                hps = prps.tile([P, D_OUT], F32, tag="hps")
                for kb in range(4):
                    nc.tensor.matmul(
                        hps[:],
                        xT[:, kb * R + it * P: kb * R + (it + 1) * P],
                        wext[:, kb * (D_OUT + 2): kb * (D_OUT + 2) + D_OUT],
                        start=(kb == 0), stop=(kb == 3))
                hsb = prsb.tile([P, D_OUT], BF16, tag="hsb")
                nc.vector.tensor_copy(hsb[:], hps[:])
                nc.scalar.dma_start(out=h_loc[it * P:(it + 1) * P, :],
                                    in_=hsb[:])
            nc.gpsimd.collective_compute(
                "AllGather", ALU.bypass, replica_groups=grp,
                ins=[h_loc[:]], outs=[h_all[:]])

        # gathered h -> SBUF with ones column (gpsimd ring: keeps the
        # collective-gated DMA off the scalar ring so queued exps don't stall;
        # chunked by jq so the first matmuls start on the first chunk)
        h_big_v = h_big[:].rearrange("p (jt c) -> p jt c", jt=NJT)
        h_all_v = h_all.rearrange("(jt p) d -> p jt d", p=P)
        for jq in range(NJQ):
            nc.gpsimd.dma_start(
                out=h_big_v[:, jq * JLQ:(jq + 1) * JLQ, 0:D_OUT],
                in_=h_all_v[:, jq * JLQ:(jq + 1) * JLQ, :])
        nc.vector.memset(h_big_v[:, :, D_OUT:HCOL], 1.0)

        # a_n broadcast across partitions (0-stride partition read from DRAM),
        # chunked so the first quarter's leaky isn't gated on the full 6.3MB
        an_bcast_src = an_all.rearrange("n one -> one n").to_broadcast((P, N))
        for jq in range(NJQ):
            nc.gpsimd.dma_start(
                out=anb[:, jq * QW:(jq + 1) * QW],
                in_=an_bcast_src[:, jq * QW:(jq + 1) * QW])

        # ---------------- main attention loop ----------------
        zpools = [octx.enter_context(tc.tile_pool(name=f"z{i}", bufs=3))
                  for i in range(ITG)]
        spool = octx.enter_context(tc.tile_pool(name="spool", bufs=3))
        mpsum = octx.enter_context(tc.tile_pool(name="mpsum", bufs=4, space="PSUM"))
        opsum = octx.enter_context(tc.tile_pool(name="opsum", bufs=1, space="PSUM"))
        fpool = octx.enter_context(tc.tile_pool(name="fpool", bufs=2))

        for g in range(GROUPS):
            outps = [opsum.tile([P, HCOL], F32, name=f"outp{i6}", tag=f"outp{i6}")
                     for i6 in range(ITG)]
            for jq in range(NJQ):
                sns = []
                for i6 in range(ITG):
                    it = g * ITG + i6
                    zt = zpools[i6].tile([P, QW], F32)
                    # adj rows, plain line-rate DMA (sync HWDGE ring)
                    nc.sync.dma_start(
                        out=zt[:],
                        in_=adj_p[it * P:(it + 1) * P, jq * QW:(jq + 1) * QW])
                    # in-place fused: zt = leaky(anb + a_s)/BIG + zt(adj)
                    nc.vector._custom_dve(
                        _LEAKY_MASK, out=zt[:],
                        in0=anb[:, jq * QW:(jq + 1) * QW], in1=zt[:],
                        s0=as_sb[:, it:it + 1], s1=0.2, imm2=1.0 / BIG)
                    # s = exp(BIG*z - BIG) natural layout, bf16
                    sn = spool.tile([P, QW], BF16, tag=f"sn{i6}")
                    nc.scalar.activation(out=sn[:], in_=zt[:], func=AFT.Exp,
                                         bias=negbig[:], scale=BIG)
                    sns.append(sn)
                for jl in range(JLQ):
                    jt = jq * JLQ + jl
                    zps = mpsum.tile([P, ITG * P], BF16)
                    for i6 in range(ITG):
                        nc.tensor.transpose(
                            zps[:, i6 * P:(i6 + 1) * P],
                            sns[i6][:, jl * P:(jl + 1) * P], ident_bf[:])
                    st = spool.tile([P, ITG * P], BF16, tag="st", bufs=4)
                    # PSUM -> SBUF copy, split over the engines with slack
                    if jl % 8 < 5:
                        nc.scalar.copy(st[:], zps[:])
                    else:
                        nc.vector.tensor_copy(st[:], zps[:])
                    for i6 in range(ITG):
                        nc.tensor.matmul(
                            outps[i6][:],
                            st[:, i6 * P:(i6 + 1) * P],
                            h_big[:, jt * HCOL:(jt + 1) * HCOL],
                            start=(jq == 0 and jl == 0),
                            stop=(jq == NJQ - 1 and jl == JLQ - 1))
            for i6 in range(ITG):
                it = g * ITG + i6
                rec = fpool.tile([P, 1], F32, tag="rec")
                nc.vector.reciprocal(rec[:], outps[i6][:, D_OUT:HCOL])
                of = fpool.tile([P, D_OUT], F32, tag="of")
                nc.vector.tensor_scalar(
                    out=of[:], in0=outps[i6][:, 0:D_OUT],
                    scalar1=rec[:], scalar2=0.0,
                    op0=ALU.mult, op1=ALU.max)
                nc.gpsimd.dma_start(out=out_p[it * P:(it + 1) * P, :], in_=of[:])

    nc.compile()
    return nc


def kernel(x, W, attn_self, attn_neigh, adj):
    global last_results
    if "nc" not in _cache:
        _cache["nc"] = _build()
    nc = _cache["nc"]

    x = np.ascontiguousarray(np.asarray(x, dtype=np.float32))
    W = np.ascontiguousarray(np.asarray(W, dtype=np.float32))
    attn_self = np.ascontiguousarray(np.asarray(attn_self, dtype=np.float32))
    attn_neigh = np.ascontiguousarray(np.asarray(attn_neigh, dtype=np.float32))
    adj = np.asarray(adj, dtype=np.float32)

    in_maps = []
    for c in range(NCORES):
        sl = slice(c * R, (c + 1) * R)
        in_maps.append({
            "x": np.ascontiguousarray(x[sl]),
            "W": W,
            "attn_self": attn_self,
            "attn_neigh": attn_neigh,
            "adj": np.ascontiguousarray(adj[sl]),
        })

    trace = bool(os.environ.get("KERNEL_TRACE"))
    if trace:
        _install_ntff_hook()
    res = run_bass_kernel_spmd(nc, in_maps, list(range(NCORES)), trace=trace)
    last_results = res
    return np.concatenate([res.results[c]["out"] for c in range(NCORES)], axis=0)
